# revision 15
# baseline (speedup 1.0000x reference)
"""PointsRenderer (alpha compositing over K points/pixel) on 8 trn2 cores.

Data-parallel over batch B=8 -> 1 image per NeuronCore. The random
per-fragment feature gather runs through the extended SWDGE dma_gather
instruction: int16 block indices j = idx>>2 into a host-expanded table
E[j] = features[4j:4j+4] (64B payload at 256B stride, the minimum the
descriptor stride encoding allows), 1024 rows per call (SWDGE ring cap),
2048 calls spread over all 4 SWDGE queues so all four Q7 core pairs
generate descriptors in parallel. The in-block 1-of-4 row selection is
two in-place DVE copy_predicated ops on host-precomputed bit masks.

Fragment layout: call t covers pixels q = t*128 + p (one pixel per
partition), slots s=0..7 = the K fragments, so compositing (weights,
front-to-back cumprod, contribution-weighted sum over K) runs on
[128, 64, 8]-shaped views per super-tile of 64 calls.

Tile's 8 DMASW semaphore lanes round-robin over Pool DMA instructions;
after scheduling, each gather's queue_num is remapped to
(position % 8)//2 so one lane always serves one queue, keeping per-lane
completion FIFO (cross-queue reordering would break WAR tracking).
"""

import numpy as np

import concourse.mybir as mybir
import concourse.tile as tile
from concourse import bacc
from concourse.bass_utils import run_bass_kernel_spmd

B, H, W, K, P, C = 8, 512, 512, 8, 100000, 4
NPIX = H * W               # 262144 pixels per core
NI = 1024                  # gather rows per call (SWDGE ring cap)
TILES = NPIX // 128        # 2048 calls per core (128 pixels per call)
ST = 64                    # calls per super-tile
NST = TILES // ST          # 32 super-tiles
S = NI // 16               # idx columns per call (16-partition wrap)
NBLK = 25000               # g=4 blocks covering 100000 rows
STEP = 64                  # f32 stride between blocks (256B)
ELEM = 16                  # f32 payload per gather (4 rows x 4 ch)

F32 = mybir.dt.float32
I16 = mybir.dt.int16
I8 = mybir.dt.int8


def dma_gather_raw(g, out_ap, in_ap, idxs_ap, num_idxs, num_idxs_reg,
                   elem_size, elem_step):
    """InstDMAGatherAnt with 64B payload (bass's dma_gather asserts 256B
    elements, a transpose-only ISA restriction; non-transpose allows any
    payload as long as the 256B-multiple stride encoding holds)."""
    stride_bytes = elem_step * mybir.dt.size(in_ap.dtype)
    assert stride_bytes % 256 == 0 and stride_bytes // 256 < 256
    return g.add_instruction(
        mybir.InstDMAGatherAnt(
            name=g.bass.get_next_instruction_name(),
            ins=[
                *g.lower_ap_dma(in_ap, for_custom_bir_dma=True),
                g.lower_ap(idxs_ap),
                g.lower_val_access(g.to_reg(num_idxs_reg)),
            ],
            outs=[g.lower_ap(out_ap)],
            transpose=False,
            num_idxs=num_idxs,
            elem_size=elem_size,
            stride_bytes_256=stride_bytes // 256,
            gen_mode=0,
            single_packet=True,
            queue_num=0,  # remapped post-schedule to align with DMASW lanes
            sbuf_tokens_per_rank=0,
            sbuf_free_dim_per_rank=0,
            sbuf_free_dim_pad_per_rank=0,
            sbuf_byte_offset=0,
        )
    )


def build(inv_r2: float, nst: int = NST, debug: bool = False,
          reps: int = 1):
    nc = bacc.Bacc(None, target_bir_lowering=False, debug=debug,
                   num_swdge_queues=4)
    E = nc.dram_tensor("E", [NBLK, STEP], F32, kind="ExternalInput")
    idxs = nc.dram_tensor("idxs", [nst, 128, ST * S], I16,
                          kind="ExternalInput")
    d2 = nc.dram_tensor("d2", [nst, 128, ST * K], F32, kind="ExternalInput")
    m0 = nc.dram_tensor("m0", [nst, 128, ST * K], I8, kind="ExternalInput")
    m1 = nc.dram_tensor("m1", [nst, 128, ST * K], I8, kind="ExternalInput")
    out = nc.dram_tensor("out", [nst, 128, ST * C], F32,
                         kind="ExternalOutput")

    with tile.TileContext(nc) as tc:
        with tc.tile_pool(name="io", bufs=3) as io, \
             tc.tile_pool(name="gp", bufs=3) as gp, \
             tc.tile_pool(name="wp", bufs=2) as wp:
            for st in [s for _ in range(reps) for s in range(nst)]:
                it = io.tile([128, ST * S], I16, tag="it")
                nc.sync.dma_start(it[:], idxs[st])
                d2t = io.tile([128, ST * K], F32, tag="d2t")
                nc.sync.dma_start(d2t[:], d2[st])
                m0t = io.tile([128, ST * K], I8, tag="m0t")
                nc.sync.dma_start(m0t[:], m0[st])
                m1t = io.tile([128, ST * K], I8, tag="m1t")
                nc.sync.dma_start(m1t[:], m1[st])

                G = gp.tile([128, ST, K, ELEM], F32, tag="G")
                for tt in range(ST):
                    # NOTE: a fresh num_idxs reg (mov) per call is required:
                    # the Q7 reads the register asynchronously at execution
                    # time, so a long-lived shared register races with
                    # physical-register reuse (observed EXEC_UNIT crash).
                    dma_gather_raw(
                        nc.gpsimd, G[:, tt], E[:, 0:ELEM],
                        it[:, tt * S:(tt + 1) * S], NI, NI, ELEM, STEP,
                    )

                # weights: alpha = 1 - d2/r^2 (ACT), om = d2/r^2 (DVE)
                alpha = wp.tile([128, ST * K], F32, tag="alpha")
                nc.scalar.activation(
                    alpha[:], d2t[:], mybir.ActivationFunctionType.Copy,
                    bias=1.0, scale=-float(inv_r2),
                )
                om = wp.tile([128, ST * K], F32, tag="om")
                nc.vector.tensor_scalar_mul(om[:], d2t[:], float(inv_r2))

                # contrib_k = alpha_k * prod_{j<k} om_j (front-to-back)
                cb = wp.tile([128, ST * K], F32, tag="cb")
                a3 = alpha[:].rearrange("p (t k) -> p t k", k=K)
                o3 = om[:].rearrange("p (t k) -> p t k", k=K)
                c3 = cb[:].rearrange("p (t k) -> p t k", k=K)
                rt = wp.tile([128, ST], F32, tag="rt")
                nc.vector.tensor_copy(c3[:, :, 0], a3[:, :, 0])
                nc.vector.tensor_copy(rt[:], o3[:, :, 0])
                for k in range(1, K):
                    nc.vector.tensor_mul(c3[:, :, k], a3[:, :, k], rt[:])
                    if k < K - 1:
                        nc.vector.tensor_mul(rt[:], rt[:], o3[:, :, k])

                # in-place 1-of-4 row select: bit1 picks half, bit0 row
                Gv = G[:]
                m1b = m1t[:].rearrange(
                    "p (t k one) -> p t k one", k=K, one=1
                ).to_broadcast([128, ST, K, 8])
                nc.vector.copy_predicated(
                    Gv[:, :, :, 0:8], m1b, Gv[:, :, :, 8:16]
                )
                m0b = m0t[:].rearrange(
                    "p (t k one) -> p t k one", k=K, one=1
                ).to_broadcast([128, ST, K, C])
                nc.vector.copy_predicated(
                    Gv[:, :, :, 0:4], m0b, Gv[:, :, :, 4:8]
                )

                # selected row *= contrib (broadcast over channel)
                Gz = Gv[:, :, :, 0:C]
                cbb = cb[:].rearrange(
                    "p (t k one) -> p t k one", k=K, one=1
                ).to_broadcast([128, ST, K, C])
                nc.vector.tensor_mul(Gz, Gz, cbb)

                # sum over K: tree reduction into compact out tile
                nc.vector.tensor_add(
                    Gz[:, :, 0:4, :], Gz[:, :, 0:4, :], Gz[:, :, 4:8, :]
                )
                nc.vector.tensor_add(
                    Gz[:, :, 0:2, :], Gz[:, :, 0:2, :], Gz[:, :, 2:4, :]
                )
                outT = wp.tile([128, ST, C], F32, tag="outT")
                nc.vector.tensor_add(
                    outT[:], Gz[:, :, 0, :], Gz[:, :, 1, :]
                )
                nc.sync.dma_start(
                    out[st], outT[:].rearrange("p t c -> p (t c)")
                )

    # Align gather queue to its Tile DMASW lane: lane = i % 8 (blind
    # round-robin over Pool DMA insts, all of which are gathers here),
    # queue = i % 4 = lane % 4 keeps each lane FIFO within one SWDGE
    # queue while rotating strictly across the four Q7 core pairs.
    i = 0
    for fn in nc.m.functions:
        for blk in fn.blocks:
            for inst in blk.instructions:
                if isinstance(inst, mybir.InstDMAGatherAnt):
                    inst.queue_num = i % 4
                    i += 1
    assert i == reps * nst * ST, f"expected {reps * nst * ST} gathers, found {i}"

    nc.compile()
    return nc


def prep_shared(features: np.ndarray) -> np.ndarray:
    """E[j] = features[4j:4j+4] flattened into the first 64B of a 256B
    stride row."""
    pad = np.zeros((4 * NBLK + 4, C), np.float32)
    pad[:P] = features
    E = np.zeros((NBLK, STEP), np.float32)
    rows = (np.arange(NBLK) * 4)[:, None] + np.arange(4)[None, :]
    E[:, 0:4 * C] = pad[rows].reshape(NBLK, 4 * C)
    return E


def prep_core(idx_b: np.ndarray, d2_b: np.ndarray, nst: int = NST):
    """Per-core input layout. idx_b int (any int dtype) [H,W,K],
    d2_b f32 [H,W,K]."""
    idx3 = np.ascontiguousarray(idx_b).reshape(NPIX, K)
    if idx3.dtype == np.int64:
        idx3 = idx3.astype(np.int32)
    j = (idx3 >> 2).astype(np.int16)          # block index < 25000
    m0v = (idx3 & 1).astype(np.int8)
    m1v = ((idx3 >> 1) & 1).astype(np.int8)
    d23 = np.ascontiguousarray(d2_b, dtype=np.float32).reshape(NPIX, K)

    npix = nst * ST * 128
    # fragment stream per call t: n = s*128 + p -> pixel q = t*128+p, k=s
    jw = j[:npix].reshape(nst * ST, 128, K).transpose(0, 2, 1)  # [T,K,128]
    # wrap each call's 1024-stream into [16, S] (n = c*16 + w)
    jw = jw.reshape(nst * ST, NI // 16, 16).transpose(0, 2, 1)  # [T,16,S]
    # super-tile columns: [nst, 16, ST*S], replicate to all 8 groups
    jw = jw.reshape(nst, ST, 16, S).transpose(0, 2, 1, 3).reshape(
        nst, 16, ST * S
    )
    idxs = np.tile(jw, (1, 8, 1))             # [nst, 128, ST*S]

    def lay(a):  # [NPIX, K] -> [nst, 128, ST*K]
        return np.ascontiguousarray(
            a[:npix].reshape(nst, ST, 128, K).transpose(0, 2, 1, 3)
            .reshape(nst, 128, ST * K)
        )

    return {
        "idxs": np.ascontiguousarray(idxs),
        "d2": lay(d23),
        "m0": lay(m0v),
        "m1": lay(m1v),
    }


def assemble(core_outs, nst: int = NST) -> np.ndarray:
    """[nst,128,ST*C] per core -> [B,H,W,C]."""
    img = np.empty((B, H, W, C), np.float32)
    for b in range(B):
        o = core_outs[b]["out"].reshape(nst, 128, ST, C)
        img[b] = o.transpose(0, 2, 1, 3).reshape(H, W, C)
    return img


def kernel(idx, dists2, features, radius):
    r = float(np.asarray(radius).reshape(-1)[0])
    inv_r2 = 1.0 / (r * r)
    nc = build(inv_r2)
    E = prep_shared(np.ascontiguousarray(features, dtype=np.float32))
    in_maps = []
    for b in range(B):
        m = prep_core(idx[b], dists2[b])
        m["E"] = E
        in_maps.append(m)
    res = run_bass_kernel_spmd(nc, in_maps, core_ids=list(range(B)))
    return assemble(res.results)


# revision 17
# speedup vs baseline: 1.7634x; 1.7634x over previous
"""PointsRenderer (alpha compositing over K points/pixel) on 8 trn2 cores.

Data-parallel over batch B=8 -> 1 image per NeuronCore. The random
per-fragment feature gather runs through the extended SWDGE dma_gather
instruction: int16 block indices j = idx>>2 into a host-expanded table
E[j] = features[4j:4j+4] (64B payload at 256B stride, the minimum the
descriptor stride encoding allows), 1024 rows per call (SWDGE ring cap),
2048 calls spread over all 4 SWDGE queues so all four Q7 core pairs
generate descriptors in parallel. The in-block 1-of-4 row selection is
two in-place DVE copy_predicated ops on host-precomputed bit masks.

Fragment layout: call t covers pixels q = t*128 + p (one pixel per
partition), slots s=0..7 = the K fragments, so compositing (weights,
front-to-back cumprod, contribution-weighted sum over K) runs on
[128, 64, 8]-shaped views per super-tile of 64 calls.

Tile's 8 DMASW semaphore lanes round-robin over Pool DMA instructions;
after scheduling, each gather's queue_num is remapped to position % 4
(= lane % 4) so one lane always serves one queue, keeping per-lane
completion FIFO (cross-queue reordering would break WAR tracking) while
rotating strictly across the four Q7 core pairs.
"""

import numpy as np

import concourse.mybir as mybir
import concourse.tile as tile
from concourse import bacc
from concourse.bass_utils import run_bass_kernel_spmd

B, H, W, K, P, C = 8, 512, 512, 8, 100000, 4
NPIX = H * W               # 262144 pixels per core
NI = 1024                  # gather rows per call (SWDGE ring cap)
TILES = NPIX // 128        # 2048 calls per core (128 pixels per call)
ST = 64                    # calls per super-tile
NST = TILES // ST          # 32 super-tiles
S = NI // 16               # idx columns per call (16-partition wrap)
NBLK = 25000               # g=4 blocks covering 100000 rows
STEP = 64                  # f32 stride between blocks (256B)
ELEM = 16                  # f32 payload per gather (4 rows x 4 ch)

F32 = mybir.dt.float32
I16 = mybir.dt.int16
I8 = mybir.dt.int8


def dma_gather_raw(g, out_ap, in_ap, idxs_ap, num_idxs, num_idxs_reg,
                   elem_size, elem_step):
    """InstDMAGatherAnt with 64B payload (bass's dma_gather asserts 256B
    elements, a transpose-only ISA restriction; non-transpose allows any
    payload as long as the 256B-multiple stride encoding holds)."""
    stride_bytes = elem_step * mybir.dt.size(in_ap.dtype)
    assert stride_bytes % 256 == 0 and stride_bytes // 256 < 256
    return g.add_instruction(
        mybir.InstDMAGatherAnt(
            name=g.bass.get_next_instruction_name(),
            ins=[
                *g.lower_ap_dma(in_ap, for_custom_bir_dma=True),
                g.lower_ap(idxs_ap),
                g.lower_val_access(g.to_reg(num_idxs_reg)),
            ],
            outs=[g.lower_ap(out_ap)],
            transpose=False,
            num_idxs=num_idxs,
            elem_size=elem_size,
            stride_bytes_256=stride_bytes // 256,
            gen_mode=0,
            single_packet=False,
            queue_num=0,  # remapped post-schedule to align with DMASW lanes
            sbuf_tokens_per_rank=0,
            sbuf_free_dim_per_rank=0,
            sbuf_free_dim_pad_per_rank=0,
            sbuf_byte_offset=0,
        )
    )


def build(inv_r2: float, nst: int = NST, debug: bool = False,
          reps: int = 1):
    nc = bacc.Bacc(None, target_bir_lowering=False, debug=debug,
                   num_swdge_queues=4)
    E = nc.dram_tensor("E", [NBLK, STEP], F32, kind="ExternalInput")
    idxs = nc.dram_tensor("idxs", [nst, 128, ST * S], I16,
                          kind="ExternalInput")
    d2 = nc.dram_tensor("d2", [nst, 128, ST * K], F32, kind="ExternalInput")
    m0 = nc.dram_tensor("m0", [nst, 128, ST * K], I8, kind="ExternalInput")
    m1 = nc.dram_tensor("m1", [nst, 128, ST * K], I8, kind="ExternalInput")
    out = nc.dram_tensor("out", [nst, 128, ST * C], F32,
                         kind="ExternalOutput")

    with tile.TileContext(nc) as tc:
        with tc.tile_pool(name="io", bufs=3) as io, \
             tc.tile_pool(name="gp", bufs=3) as gp, \
             tc.tile_pool(name="wp", bufs=2) as wp:
            for st in [s for _ in range(reps) for s in range(nst)]:
                it = io.tile([128, ST * S], I16, tag="it")
                nc.sync.dma_start(it[:], idxs[st])
                d2t = io.tile([128, ST * K], F32, tag="d2t")
                nc.sync.dma_start(d2t[:], d2[st])
                m0t = io.tile([128, ST * K], I8, tag="m0t")
                nc.sync.dma_start(m0t[:], m0[st])
                m1t = io.tile([128, ST * K], I8, tag="m1t")
                nc.sync.dma_start(m1t[:], m1[st])

                G = gp.tile([128, ST, K, ELEM], F32, tag="G")
                for tt in range(ST):
                    # NOTE: a fresh num_idxs reg (mov) per call is required:
                    # the Q7 reads the register asynchronously at execution
                    # time, so a long-lived shared register races with
                    # physical-register reuse (observed EXEC_UNIT crash).
                    dma_gather_raw(
                        nc.gpsimd, G[:, tt], E[:, 0:ELEM],
                        it[:, tt * S:(tt + 1) * S], NI, NI, ELEM, STEP,
                    )

                # weights: alpha = 1 - d2/r^2 (ACT), om = d2/r^2 (DVE)
                alpha = wp.tile([128, ST * K], F32, tag="alpha")
                nc.scalar.activation(
                    alpha[:], d2t[:], mybir.ActivationFunctionType.Copy,
                    bias=1.0, scale=-float(inv_r2),
                )
                om = wp.tile([128, ST * K], F32, tag="om")
                nc.vector.tensor_scalar_mul(om[:], d2t[:], float(inv_r2))

                # contrib_k = alpha_k * prod_{j<k} om_j (front-to-back)
                cb = wp.tile([128, ST * K], F32, tag="cb")
                a3 = alpha[:].rearrange("p (t k) -> p t k", k=K)
                o3 = om[:].rearrange("p (t k) -> p t k", k=K)
                c3 = cb[:].rearrange("p (t k) -> p t k", k=K)
                rt = wp.tile([128, ST], F32, tag="rt")
                nc.vector.tensor_copy(c3[:, :, 0], a3[:, :, 0])
                nc.vector.tensor_copy(rt[:], o3[:, :, 0])
                for k in range(1, K):
                    nc.vector.tensor_mul(c3[:, :, k], a3[:, :, k], rt[:])
                    if k < K - 1:
                        nc.vector.tensor_mul(rt[:], rt[:], o3[:, :, k])

                # in-place 1-of-4 row select: bit1 picks half, bit0 row
                Gv = G[:]
                m1b = m1t[:].rearrange(
                    "p (t k one) -> p t k one", k=K, one=1
                ).to_broadcast([128, ST, K, 8])
                nc.vector.copy_predicated(
                    Gv[:, :, :, 0:8], m1b, Gv[:, :, :, 8:16]
                )
                m0b = m0t[:].rearrange(
                    "p (t k one) -> p t k one", k=K, one=1
                ).to_broadcast([128, ST, K, C])
                nc.vector.copy_predicated(
                    Gv[:, :, :, 0:4], m0b, Gv[:, :, :, 4:8]
                )

                # selected row *= contrib (broadcast over channel)
                Gz = Gv[:, :, :, 0:C]
                cbb = cb[:].rearrange(
                    "p (t k one) -> p t k one", k=K, one=1
                ).to_broadcast([128, ST, K, C])
                nc.vector.tensor_mul(Gz, Gz, cbb)

                # sum over K: tree reduction into compact out tile
                nc.vector.tensor_add(
                    Gz[:, :, 0:4, :], Gz[:, :, 0:4, :], Gz[:, :, 4:8, :]
                )
                nc.vector.tensor_add(
                    Gz[:, :, 0:2, :], Gz[:, :, 0:2, :], Gz[:, :, 2:4, :]
                )
                outT = wp.tile([128, ST, C], F32, tag="outT")
                nc.vector.tensor_add(
                    outT[:], Gz[:, :, 0, :], Gz[:, :, 1, :]
                )
                nc.sync.dma_start(
                    out[st], outT[:].rearrange("p t c -> p (t c)")
                )

    # Align gather queue to its Tile DMASW lane: lane = i % 8 (blind
    # round-robin over Pool DMA insts, all of which are gathers here),
    # queue = i % 4 = lane % 4 keeps each lane FIFO within one SWDGE
    # queue while rotating strictly across the four Q7 core pairs.
    i = 0
    for fn in nc.m.functions:
        for blk in fn.blocks:
            for inst in blk.instructions:
                if isinstance(inst, mybir.InstDMAGatherAnt):
                    inst.queue_num = i % 4
                    i += 1
    assert i == reps * nst * ST, f"expected {reps * nst * ST} gathers, found {i}"

    nc.compile()
    return nc


def prep_shared(features: np.ndarray) -> np.ndarray:
    """E[j] = features[4j:4j+4] flattened into the first 64B of a 256B
    stride row."""
    pad = np.zeros((4 * NBLK + 4, C), np.float32)
    pad[:P] = features
    E = np.zeros((NBLK, STEP), np.float32)
    rows = (np.arange(NBLK) * 4)[:, None] + np.arange(4)[None, :]
    E[:, 0:4 * C] = pad[rows].reshape(NBLK, 4 * C)
    return E


def prep_core(idx_b: np.ndarray, d2_b: np.ndarray, nst: int = NST):
    """Per-core input layout. idx_b int (any int dtype) [H,W,K],
    d2_b f32 [H,W,K]."""
    idx3 = np.ascontiguousarray(idx_b).reshape(NPIX, K)
    if idx3.dtype == np.int64:
        idx3 = idx3.astype(np.int32)
    j = (idx3 >> 2).astype(np.int16)          # block index < 25000
    m0v = (idx3 & 1).astype(np.int8)
    m1v = ((idx3 >> 1) & 1).astype(np.int8)
    d23 = np.ascontiguousarray(d2_b, dtype=np.float32).reshape(NPIX, K)

    npix = nst * ST * 128
    # fragment stream per call t: n = s*128 + p -> pixel q = t*128+p, k=s
    jw = j[:npix].reshape(nst * ST, 128, K).transpose(0, 2, 1)  # [T,K,128]
    # wrap each call's 1024-stream into [16, S] (n = c*16 + w)
    jw = jw.reshape(nst * ST, NI // 16, 16).transpose(0, 2, 1)  # [T,16,S]
    # super-tile columns: [nst, 16, ST*S], replicate to all 8 groups
    jw = jw.reshape(nst, ST, 16, S).transpose(0, 2, 1, 3).reshape(
        nst, 16, ST * S
    )
    idxs = np.tile(jw, (1, 8, 1))             # [nst, 128, ST*S]

    def lay(a):  # [NPIX, K] -> [nst, 128, ST*K]
        return np.ascontiguousarray(
            a[:npix].reshape(nst, ST, 128, K).transpose(0, 2, 1, 3)
            .reshape(nst, 128, ST * K)
        )

    return {
        "idxs": np.ascontiguousarray(idxs),
        "d2": lay(d23),
        "m0": lay(m0v),
        "m1": lay(m1v),
    }


def assemble(core_outs, nst: int = NST) -> np.ndarray:
    """[nst,128,ST*C] per core -> [B,H,W,C]."""
    img = np.empty((B, H, W, C), np.float32)
    for b in range(B):
        o = core_outs[b]["out"].reshape(nst, 128, ST, C)
        img[b] = o.transpose(0, 2, 1, 3).reshape(H, W, C)
    return img


def kernel(idx, dists2, features, radius):
    r = float(np.asarray(radius).reshape(-1)[0])
    inv_r2 = 1.0 / (r * r)
    nc = build(inv_r2)
    E = prep_shared(np.ascontiguousarray(features, dtype=np.float32))
    in_maps = []
    for b in range(B):
        m = prep_core(idx[b], dists2[b])
        m["E"] = E
        in_maps.append(m)
    res = run_bass_kernel_spmd(nc, in_maps, core_ids=list(range(B)))
    return assemble(res.results)
